# revision 14
# baseline (speedup 1.0000x reference)
"""CoAttention forward on 8 TRN2 NeuronCores.

Data-parallel over batch B=64 (8 batches/core). All precision-critical
matmuls are 3-pass f16 hi/lo (~22-bit); the softmax logits are extremely
sensitive (pre-tanh affinities have std ~1e3, softmax is near-argmax), so
2-pass is not enough on the affinity/projection paths.

Structure (per batch, Q [512,1024], V [196,1024], D=1024):
  Bv   = W_b V^T               [D, NV]   3-pass, split hi/lo on chip
  C    = tanh(Q Bv)            [NQ, NV]  3-pass, stored f16
         (C = Q (W_b V^T) association halves the affinity-path PE work
          vs (Q W_b) V^T since NV=196 < NQ=512)
  WqQT = Q W_q^T               [NQ, D]   3-pass, split hi/lo
  WvVT = V W_v^T               [NV, D]   3-pass, split hi/lo
  CT   = C^T                   via PE f16 transposes (after W so tanh overlaps)
  H_v  = tanh(transpose(WvVT_hi) + (WqQT_hi + WqQT_lo) C)   f16 store
  h_v  = [whv_hi whv_lo]^T H_v   (M=2 matmul; rows summed on GpSimd)
  a_v  = softmax(h_v); broadcast to 128 partitions via f16 PE matmul
  H_q / h_q / a_q analogous with CT
  v_hat = sum_v a_v[v] VT_hi[:, v];  q_hat = sum_q a_q[q] QT_hi[:, q]

Cross-batch software pipelining: batch b's q-softmax + v_hat/q_hat
reductions + output DMA are emitted inside batch b+1's Bv phase; the
v-softmax is woven into the H_q loop. This keeps the PE from stalling on
the vector/scalar softmax chains at batch boundaries.

kernel(**inputs) takes FULL inputs, shards internally, returns (v_hat, q_hat).
"""
import numpy as np

import concourse.bass as bass
import concourse.mybir as mybir
import concourse.tile as tile
from concourse import bacc, bass_isa
from concourse.bass_utils import run_bass_kernel_spmd
from concourse.masks import make_identity

AF = mybir.ActivationFunctionType
ALU = mybir.AluOpType
AX = mybir.AxisListType
F32 = mybir.dt.float32
F16 = mybir.dt.float16

B, NV, NQ, D = 64, 196, 512, 1024
NCORES = 8
NB = B // NCORES          # batches per core
KD = D // 128             # 8 feature k-tiles
MQ = NQ // 128            # 4 NQ m-tiles
NV1 = NV - 128            # 68 (second NV tile)


def build(nb=NB):
    nc = bacc.Bacc(None, target_bir_lowering=False)

    QTh_d = nc.dram_tensor("QTh", [nb, D, NQ], F16, kind="ExternalInput")
    QTl_d = nc.dram_tensor("QTl", [nb, D, NQ], F16, kind="ExternalInput")
    VTh_d = nc.dram_tensor("VTh", [nb, D, NV], F16, kind="ExternalInput")
    VTl_d = nc.dram_tensor("VTl", [nb, D, NV], F16, kind="ExternalInput")
    WbTh_d = nc.dram_tensor("WbTh", [D, D], F16, kind="ExternalInput")
    WbTl_d = nc.dram_tensor("WbTl", [D, D], F16, kind="ExternalInput")
    WqTh_d = nc.dram_tensor("WqTh", [D, D], F16, kind="ExternalInput")
    WqTl_d = nc.dram_tensor("WqTl", [D, D], F16, kind="ExternalInput")
    WvTh_d = nc.dram_tensor("WvTh", [D, D], F16, kind="ExternalInput")
    WvTl_d = nc.dram_tensor("WvTl", [D, D], F16, kind="ExternalInput")
    whv_d = nc.dram_tensor("whv2", [D, 2], F16, kind="ExternalInput")
    whq_d = nc.dram_tensor("whq2", [D, 2], F16, kind="ExternalInput")
    OV_d = nc.dram_tensor("OV", [nb, D], F32, kind="ExternalOutput")
    OQ_d = nc.dram_tensor("OQ", [nb, D], F32, kind="ExternalOutput")

    with tile.TileContext(nc) as tc:
        with (
            tc.tile_pool(name="wsb", bufs=1) as wsb,
            tc.tile_pool(name="iop", bufs=2) as iop,
            tc.tile_pool(name="mid", bufs=1) as mid,
            tc.tile_pool(name="sm", bufs=1) as sm,
            tc.tile_pool(name="psp", bufs=4, space="PSUM") as psp,
        ):
            def wtile(name, src, split=False):
                t = wsb.tile([128, KD, D], F16, name=name)
                ap = src.rearrange("(k p) d -> p k d", p=128)
                if split:
                    # per-k-tile transfers: first Bv matmuls only gate on k=0
                    for k in range(KD):
                        nc.sync.dma_start(out=t[:, k, :], in_=ap[:, k, :])
                else:
                    nc.sync.dma_start(out=t, in_=ap)
                return t

            def load_inputs(b, split=False):
                # V first: the Bv chains consume it before Q is needed
                def inp(name, src, n):
                    t = iop.tile([128, KD, n], F16, tag=name)
                    ap = src.rearrange("(k p) n -> p k n", p=128)
                    if split:
                        for k in range(KD):
                            nc.sync.dma_start(out=t[:, k, :], in_=ap[:, k, :])
                    else:
                        nc.sync.dma_start(out=t, in_=ap)
                    return t

                vth = inp("vth", VTh_d[b], NV)
                vtl = inp("vtl", VTl_d[b], NV)
                qth = inp("qth", QTh_d[b], NQ)
                qtl = inp("qtl", QTl_d[b], NQ)
                return qth, qtl, vth, vtl

            # DMA order = queue order: Bv-critical tensors first so the PE
            # can start ~20us after launch instead of waiting on all weights.
            wbth = wtile("wbth", WbTh_d, split=True)
            wbtl = wtile("wbtl", WbTl_d, split=True)
            pre0 = load_inputs(0, split=True)
            wqth = wtile("wqth", WqTh_d)
            wqtl = wtile("wqtl", WqTl_d)
            wvth = wtile("wvth", WvTh_d)
            wvtl = wtile("wvtl", WvTl_d)
            whv_sb = wsb.tile([128, KD, 2], F16)
            nc.sync.dma_start(out=whv_sb, in_=whv_d.rearrange("(k p) t -> p k t", p=128))
            whq_sb = wsb.tile([128, KD, 2], F16)
            nc.sync.dma_start(out=whq_sb, in_=whq_d.rearrange("(k p) t -> p k t", p=128))
            identh = wsb.tile([128, 128], F16)
            make_identity(nc, identh)
            ones_row = wsb.tile([1, 128], F32)
            nc.vector.memset(ones_row, 1.0)

            def softmax_bcast(b, h_ps, n, tagp):
                negm = sm.tile([1, 1], F32, tag=f"negm{tagp}")
                nc.vector.reduce_max(negm, h_ps, axis=AX.X, negate=True)
                ex = sm.tile([1, n], F16, tag=f"ex{tagp}")
                ssum = sm.tile([1, 1], F32, tag=f"ssum{tagp}")
                nc.scalar.activation(ex, h_ps, AF.Exp, bias=negm, accum_out=ssum)
                rs = sm.tile([1, 1], F32, tag=f"rs{tagp}")
                nc.vector.reciprocal(rs, ssum)
                ones_s = sm.tile([1, 128], F16, tag=f"ones_s{tagp}")
                nc.vector.tensor_scalar_mul(ones_s, ones_row, rs)
                ab_ps = psp.tile([128, n], F32, tag="ps512", bufs=4, name=f"abps{tagp}{b}")
                nc.tensor.matmul(ab_ps, ones_s, ex, start=True, stop=True)
                ab = sm.tile([128, n], F16, tag=f"ab{tagp}")
                nc.scalar.copy(ab, ab_ps)
                return ab

            tail_fn = [None]

            for b in range(nb):
                qth, qtl, vth, vtl = pre0 if b == 0 else load_inputs(b)

                # ---- phase 1+2 interleaved: Bv = Wb V^T, C = Q Bv (3-pass) ----
                bv_hi = mid.tile([128, KD, NV], F16, tag="bv_hi")
                bv_lo = mid.tile([128, KD, NV], F16, tag="bv_lo")
                c_ps = [psp.tile([128, NV], F32, tag="ps196", name=f"c_ps{b}_{m}")
                        for m in range(MQ)]

                def emit_bv(md):
                    pb = psp.tile([128, NV], F32, tag="ps512", bufs=4, name=f"pb{b}_{md}")
                    passes = ((wbth, vth), (wbth, vtl), (wbtl, vth))
                    ds = slice(md * 128, (md + 1) * 128)
                    n = 0
                    for k in range(KD):
                        for lh, rh in passes:
                            n += 1
                            nc.tensor.matmul(pb, lh[:, k, ds], rh[:, k, :],
                                             start=(n == 1), stop=(n == 3 * KD))
                    nc.vector.tensor_copy(bv_hi[:, md, :], pb)
                    nc.vector.tensor_sub(bv_lo[:, md, :], pb, bv_hi[:, md, :])

                def emit_c(k):
                    for m in range(MQ):
                        ms = slice(m * 128, (m + 1) * 128)
                        for i, (lh, rh) in enumerate(((qth, bv_hi), (qtl, bv_hi), (qth, bv_lo))):
                            nc.tensor.matmul(c_ps[m], lh[:, k, ms], rh[:, k, :],
                                             start=(k == 0 and i == 0),
                                             stop=(k == KD - 1 and i == 2))

                for e in range(KD + 1):
                    if e < KD:
                        emit_bv(e)
                    if e == 2 and tail_fn[0] is not None:
                        tail_fn[0]()          # batch b-1 softmax_q + outputs
                        tail_fn[0] = None
                    if e >= 1:
                        emit_c(e - 1)

                c_sb = mid.tile([128, MQ, NV], F16, tag="c")
                for m in range(MQ):
                    nc.scalar.activation(c_sb[:, m, :], c_ps[m], AF.Tanh)

                # ---- phase 3: WqQT, WvVT (3-pass, split hi/lo) ----
                wqqt_hi = mid.tile([128, MQ, D], F16, tag="wqqt_hi")
                wqqt_lo = mid.tile([128, MQ, D], F16, tag="wqqt_lo")
                for m in range(MQ):
                    ms = slice(m * 128, (m + 1) * 128)
                    for h in range(2):
                        hs = slice(h * 512, (h + 1) * 512)
                        p = psp.tile([128, 512], F32, tag="ps512", bufs=4, name=f"pq{b}_{m}_{h}")
                        n = 0
                        for k in range(KD):
                            for lh, rh in ((qth, wqth), (qth, wqtl), (qtl, wqth)):
                                n += 1
                                nc.tensor.matmul(p, lh[:, k, ms], rh[:, k, hs],
                                                 start=(n == 1), stop=(n == 3 * KD))
                        nc.vector.tensor_copy(wqqt_hi[:, m, hs], p)
                        nc.vector.tensor_sub(wqqt_lo[:, m, hs], p, wqqt_hi[:, m, hs])
                wvvt_hi = mid.tile([128, 2, D], F16, tag="wvvt_hi")
                wvvt_lo = mid.tile([128, 2, D], F16, tag="wvvt_lo")
                for m in range(2):
                    rows = 128 if m == 0 else NV1
                    ms = slice(m * 128, m * 128 + rows)
                    for h in range(2):
                        hs = slice(h * 512, (h + 1) * 512)
                        p = psp.tile([128, 512], F32, tag="ps512", bufs=4, name=f"pv{b}_{m}_{h}")
                        n = 0
                        for k in range(KD):
                            for lh, rh in ((vth, wvth), (vth, wvtl), (vtl, wvth)):
                                n += 1
                                nc.tensor.matmul(p[:rows, :], lh[:, k, ms], rh[:, k, hs],
                                                 start=(n == 1), stop=(n == 3 * KD))
                        nc.vector.tensor_copy(wvvt_hi[:rows, m, hs], p[:rows, :])
                        nc.vector.tensor_sub(wvvt_lo[:rows, m, hs], p[:rows, :],
                                             wvvt_hi[:rows, m, hs])

                # ---- CT via f16 PE transposes (tanh of C overlapped W above) ----
                ct_sb = mid.tile([128, 2, NQ], F16, tag="ct")
                for mv in range(2):
                    rows = 128 if mv == 0 else NV1
                    ctp = psp.tile([128, NQ], F16, tag="ps512", bufs=4, name=f"ctp{b}_{mv}")
                    for mq in range(MQ):
                        nc.tensor.matmul(
                            ctp[:rows, mq * 128:(mq + 1) * 128],
                            c_sb[:, mq, mv * 128:mv * 128 + rows],
                            identh, is_transpose=True,
                            start=(mq == 0), stop=(mq == MQ - 1))
                    nc.scalar.copy(ct_sb[:rows, mv, :], ctp[:rows, :])

                # ---- phase 4: H_v (f16) + h_v [2-row dot] ----
                hv_m_l = [None] * KD
                h_v_ps = psp.tile([1, NV], F32, tag="ps196", name=f"hv_acc{b}")

                def emit_hv(m):
                    ms = slice(m * 128, (m + 1) * 128)
                    t2 = psp.tile([128, NV], F32, tag="ps196", name=f"hv2_{b}_{m}")
                    for kq in range(MQ):
                        for i, lh in enumerate((wqqt_hi, wqqt_lo)):
                            nc.tensor.matmul(t2, lh[:, kq, ms], c_sb[:, kq, :],
                                             start=(kq == 0 and i == 0),
                                             stop=(kq == MQ - 1 and i == 1))
                    t1 = psp.tile([128, NV], F16, tag="ps196", name=f"hv1_{b}_{m}")
                    nc.tensor.matmul(t1[:, 0:128], wvvt_hi[:, 0, ms], identh,
                                     is_transpose=True, start=True, stop=False)
                    nc.tensor.matmul(t1[:, 128:NV], wvvt_hi[:NV1, 1, ms],
                                     identh[:NV1, :NV1],
                                     is_transpose=True, start=False, stop=True)
                    t1sb = sm.tile([128, NV], F16, tag="t1v", bufs=2, name=f"t1v{b}_{m}")
                    nc.scalar.copy(t1sb, t1)
                    pre = sm.tile([128, NV], F32, tag="prev", bufs=2, name=f"prev{b}_{m}")
                    nc.vector.scalar_tensor_tensor(out=pre, in0=t2, scalar=1.0, in1=t1sb,
                                                   op0=ALU.mult, op1=ALU.add)
                    hv_m = sm.tile([128, NV], F16, tag="hvm", bufs=2, name=f"hvm{b}_{m}")
                    nc.scalar.activation(hv_m, pre, AF.Tanh)
                    hv_m_l[m] = hv_m

                def emit_hv_dot(m):
                    nc.tensor.matmul(h_v_ps, whv_sb[:, m, 0:1], hv_m_l[m],
                                     start=(m == 0), stop=False)
                    nc.tensor.matmul(h_v_ps, whv_sb[:, m, 1:2], hv_m_l[m],
                                     start=False, stop=(m == KD - 1))

                for m in range(KD + 1):
                    if m < KD:
                        emit_hv(m)
                    if m >= 1:
                        emit_hv_dot(m - 1)

                # ---- phase 5: H_q + h_q, with v-softmax/v_hat woven in ----
                hq_m_l = [None] * KD
                h_q_ps = psp.tile([1, NQ], F32, tag="ps512", bufs=4, name=f"hq_acc{b}")
                av_b_box = [None]
                vhat_sb = sm.tile([128, KD], F32, tag="vhat")

                def emit_hq(m):
                    ms = slice(m * 128, (m + 1) * 128)
                    t2 = psp.tile([128, NQ], F32, tag="ps512", bufs=4, name=f"hq2_{b}_{m}")
                    for kv in range(2):
                        rows = 128 if kv == 0 else NV1
                        for i, lh in enumerate((wvvt_hi, wvvt_lo)):
                            nc.tensor.matmul(t2, lh[:rows, kv, ms], ct_sb[:rows, kv, :],
                                             start=(kv == 0 and i == 0),
                                             stop=(kv == 1 and i == 1))
                    t1 = psp.tile([128, NQ], F16, tag="ps512", bufs=4, name=f"hq1_{b}_{m}")
                    for kq in range(MQ):
                        nc.tensor.matmul(t1[:, kq * 128:(kq + 1) * 128],
                                         wqqt_hi[:, kq, ms], identh, is_transpose=True,
                                         start=(kq == 0), stop=(kq == MQ - 1))
                    t1sb = sm.tile([128, NQ], F16, tag="t1q", bufs=2, name=f"t1q{b}_{m}")
                    nc.scalar.copy(t1sb, t1)
                    pre = sm.tile([128, NQ], F32, tag="preq", bufs=2, name=f"preq{b}_{m}")
                    nc.vector.scalar_tensor_tensor(out=pre, in0=t2, scalar=1.0, in1=t1sb,
                                                   op0=ALU.mult, op1=ALU.add)
                    hq_m = sm.tile([128, NQ], F16, tag="hqm", bufs=2, name=f"hqm{b}_{m}")
                    nc.scalar.activation(hq_m, pre, AF.Tanh)
                    hq_m_l[m] = hq_m

                def emit_hq_dot(m):
                    nc.tensor.matmul(h_q_ps, whq_sb[:, m, 0:1], hq_m_l[m],
                                     start=(m == 0), stop=False)
                    nc.tensor.matmul(h_q_ps, whq_sb[:, m, 1:2], hq_m_l[m],
                                     start=False, stop=(m == KD - 1))

                for m in range(KD + 1):
                    if m < KD:
                        emit_hq(m)
                    if m == 1:
                        av_b_box[0] = softmax_bcast(b, h_v_ps, NV, "v")
                    if m == 2:
                        for k in range(KD):
                            nc.vector.scalar_tensor_tensor(
                                out=sm.tile([128, NV], F16, tag="scrv", name="scrv"),
                                in0=vth[:, k, :], scalar=1.0, in1=av_b_box[0],
                                op0=ALU.mult, op1=ALU.mult,
                                accum_out=vhat_sb[:, k:k + 1])
                    if m >= 1:
                        emit_hq_dot(m - 1)

                # ---- deferred tail: q-softmax + q_hat + output DMA ----
                def make_tail(b, h_q_ps, qth, vhat_sb):
                    def tail():
                        aq_b = softmax_bcast(b, h_q_ps, NQ, "q")
                        qhat_sb = sm.tile([128, KD], F32, tag="qhat")
                        for k in range(KD):
                            nc.vector.scalar_tensor_tensor(
                                out=sm.tile([128, NQ], F16, tag="scrq", name="scrq"),
                                in0=qth[:, k, :], scalar=1.0, in1=aq_b,
                                op0=ALU.mult, op1=ALU.mult,
                                accum_out=qhat_sb[:, k:k + 1])
                        nc.sync.dma_start(out=OV_d[b].rearrange("(k p) -> p k", p=128),
                                          in_=vhat_sb)
                        nc.sync.dma_start(out=OQ_d[b].rearrange("(k p) -> p k", p=128),
                                          in_=qhat_sb)
                    return tail

                tail_fn[0] = make_tail(b, h_q_ps, qth, vhat_sb)

            tail_fn[0]()

    nc.finalize()
    return nc


_BUILT = {}


def _split(x):
    hi = x.astype(np.float16)
    lo = (x - hi.astype(np.float32)).astype(np.float16)
    return np.ascontiguousarray(hi), np.ascontiguousarray(lo)


def kernel(V, Q, W_b, W_v, W_q, w_hv, w_hq, _trace=False):
    V = np.asarray(V, dtype=np.float32)
    Q = np.asarray(Q, dtype=np.float32)
    nb = B // NCORES
    QTh, QTl = _split(Q.transpose(0, 2, 1))      # [B, D, NQ] f16
    VTh, VTl = _split(V.transpose(0, 2, 1))      # [B, D, NV] f16
    WbTh, WbTl = _split(np.asarray(W_b, dtype=np.float32).T)
    WqTh, WqTl = _split(np.asarray(W_q, dtype=np.float32).T)
    WvTh, WvTl = _split(np.asarray(W_v, dtype=np.float32).T)

    def whsplit(w):
        w = np.asarray(w, dtype=np.float32).reshape(D)
        hi = w.astype(np.float16)
        lo = (w - hi.astype(np.float32)).astype(np.float16)
        return np.ascontiguousarray(np.stack([hi, lo], axis=-1))  # [D, 2] f16

    whv2 = whsplit(w_hv)
    whq2 = whsplit(w_hq)

    if nb not in _BUILT:
        _BUILT[nb] = build(nb)
    nc = _BUILT[nb]

    in_maps = []
    for c in range(NCORES):
        sl = slice(c * nb, (c + 1) * nb)
        in_maps.append({
            "QTh": np.ascontiguousarray(QTh[sl]), "QTl": np.ascontiguousarray(QTl[sl]),
            "VTh": np.ascontiguousarray(VTh[sl]), "VTl": np.ascontiguousarray(VTl[sl]),
            "WbTh": WbTh, "WbTl": WbTl, "WqTh": WqTh, "WqTl": WqTl,
            "WvTh": WvTh, "WvTl": WvTl, "whv2": whv2, "whq2": whq2,
        })

    out = run_bass_kernel_spmd(nc, in_maps, core_ids=list(range(NCORES)),
                               trace=_trace)
    v_hat = np.concatenate([out.results[c]["OV"] for c in range(NCORES)], axis=0)
    q_hat = np.concatenate([out.results[c]["OQ"] for c in range(NCORES)], axis=0)
    if _trace:
        kernel._last_exec_ns = out.exec_time_ns
        kernel._last_results = out
    return (v_hat, q_hat)


# revision 20
# speedup vs baseline: 1.0605x; 1.0605x over previous
"""CoAttention forward on 8 TRN2 NeuronCores.

Data-parallel over batch B=64 (8 batches/core). All precision-critical
matmuls are 3-pass f16 hi/lo (~22-bit); the softmax logits are extremely
sensitive (pre-tanh affinities have std ~1e3, softmax is near-argmax), so
2-pass is not enough on the affinity/projection paths.

Structure (per batch, Q [512,1024], V [196,1024], D=1024):
  Bv   = W_b V^T               [D, NV]   3-pass, split hi/lo on chip
  C    = tanh(Q Bv)            [NQ, NV]  3-pass, stored f16
         (C = Q (W_b V^T) association halves the affinity-path PE work
          vs (Q W_b) V^T since NV=196 < NQ=512)
  WqQT = Q W_q^T               [NQ, D]   3-pass, split hi/lo
  WvVT = V W_v^T               [NV, D]   3-pass, split hi/lo
  CT   = C^T                   via PE f16 transposes (after W so tanh overlaps)
  H_v  = tanh(transpose(WvVT_hi) + (WqQT_hi + WqQT_lo) C)   f16 store
  h_v  = [whv_hi whv_lo]^T H_v   (M=2 matmul; rows summed on GpSimd)
  a_v  = softmax(h_v); broadcast to 128 partitions via f16 PE matmul
  H_q / h_q / a_q analogous with CT
  v_hat = sum_v a_v[v] VT_hi[:, v];  q_hat = sum_q a_q[q] QT_hi[:, q]

Cross-batch software pipelining: batch b's q-softmax + v_hat/q_hat
reductions + output DMA are emitted inside batch b+1's Bv phase; the
v-softmax is woven into the H_q loop. This keeps the PE from stalling on
the vector/scalar softmax chains at batch boundaries.

kernel(**inputs) takes FULL inputs, shards internally, returns (v_hat, q_hat).
"""
import numpy as np

import concourse.bass as bass
import concourse.mybir as mybir
import concourse.tile as tile
from concourse import bacc, bass_isa
from concourse.bass_utils import run_bass_kernel_spmd
from concourse.masks import make_identity

AF = mybir.ActivationFunctionType
ALU = mybir.AluOpType
AX = mybir.AxisListType
F32 = mybir.dt.float32
F16 = mybir.dt.float16

B, NV, NQ, D = 64, 196, 512, 1024
NCORES = 8
NB = B // NCORES          # batches per core
KD = D // 128             # 8 feature k-tiles
MQ = NQ // 128            # 4 NQ m-tiles
NV1 = NV - 128            # 68 (second NV tile)


def build(nb=NB):
    nc = bacc.Bacc(None, target_bir_lowering=False)

    QTh_d = nc.dram_tensor("QTh", [nb, D, NQ], F16, kind="ExternalInput")
    QTl_d = nc.dram_tensor("QTl", [nb, D, NQ], F16, kind="ExternalInput")
    VTh_d = nc.dram_tensor("VTh", [nb, D, NV], F16, kind="ExternalInput")
    VTl_d = nc.dram_tensor("VTl", [nb, D, NV], F16, kind="ExternalInput")
    WbTh_d = nc.dram_tensor("WbTh", [D, D], F16, kind="ExternalInput")
    WbTl_d = nc.dram_tensor("WbTl", [D, D], F16, kind="ExternalInput")
    WqTh_d = nc.dram_tensor("WqTh", [D, D], F16, kind="ExternalInput")
    WqTl_d = nc.dram_tensor("WqTl", [D, D], F16, kind="ExternalInput")
    WvTh_d = nc.dram_tensor("WvTh", [D, D], F16, kind="ExternalInput")
    WvTl_d = nc.dram_tensor("WvTl", [D, D], F16, kind="ExternalInput")
    whv_d = nc.dram_tensor("whv2", [D, 2], F16, kind="ExternalInput")
    whq_d = nc.dram_tensor("whq2", [D, 2], F16, kind="ExternalInput")
    OV_d = nc.dram_tensor("OV", [nb, D], F32, kind="ExternalOutput")
    OQ_d = nc.dram_tensor("OQ", [nb, D], F32, kind="ExternalOutput")

    with tile.TileContext(nc) as tc:
        with (
            tc.tile_pool(name="wsb", bufs=1) as wsb,
            tc.tile_pool(name="iop", bufs=2) as iop,
            tc.tile_pool(name="mid", bufs=1) as mid,
            tc.tile_pool(name="sm", bufs=1) as sm,
            tc.tile_pool(name="psp", bufs=4, space="PSUM") as psp,
        ):
            def wtile(name, src, split=False):
                t = wsb.tile([128, KD, D], F16, name=name)
                ap = src.rearrange("(k p) d -> p k d", p=128)
                if split:
                    # per-k-tile transfers: first Bv matmuls only gate on k=0
                    for k in range(KD):
                        nc.sync.dma_start(out=t[:, k, :], in_=ap[:, k, :])
                else:
                    nc.sync.dma_start(out=t, in_=ap)
                return t

            def load_inputs(b, split=False):
                # V first: the Bv chains consume it before Q is needed
                def inp(name, src, n):
                    t = iop.tile([128, KD, n], F16, tag=name)
                    ap = src.rearrange("(k p) n -> p k n", p=128)
                    if split:
                        for k in range(KD):
                            nc.sync.dma_start(out=t[:, k, :], in_=ap[:, k, :])
                    else:
                        nc.sync.dma_start(out=t, in_=ap)
                    return t

                vth = inp("vth", VTh_d[b], NV)
                vtl = inp("vtl", VTl_d[b], NV)
                qth = inp("qth", QTh_d[b], NQ)
                qtl = inp("qtl", QTl_d[b], NQ)
                return qth, qtl, vth, vtl

            # DMA order = queue order: Bv-critical tensors first so the PE
            # can start ~20us after launch instead of waiting on all weights.
            wbth = wtile("wbth", WbTh_d, split=True)
            wbtl = wtile("wbtl", WbTl_d, split=True)
            pre0 = load_inputs(0, split=True)
            wqth = wtile("wqth", WqTh_d)
            wqtl = wtile("wqtl", WqTl_d)
            wvth = wtile("wvth", WvTh_d)
            wvtl = wtile("wvtl", WvTl_d)
            whv_sb = wsb.tile([128, KD, 2], F16)
            nc.sync.dma_start(out=whv_sb, in_=whv_d.rearrange("(k p) t -> p k t", p=128))
            whq_sb = wsb.tile([128, KD, 2], F16)
            nc.sync.dma_start(out=whq_sb, in_=whq_d.rearrange("(k p) t -> p k t", p=128))
            identh = wsb.tile([128, 128], F16)
            make_identity(nc, identh)
            ones_row = wsb.tile([1, 128], F32)
            nc.vector.memset(ones_row, 1.0)

            def softmax_bcast(b, h_sb, n, tagp):
                negm = sm.tile([1, 1], F32, tag=f"negm{tagp}")
                nc.vector.reduce_max(negm, h_sb, axis=AX.X, negate=True)
                ex = sm.tile([1, n], F16, tag=f"ex{tagp}")
                ssum = sm.tile([1, 1], F32, tag=f"ssum{tagp}")
                nc.scalar.activation(ex, h_sb, AF.Exp, bias=negm, accum_out=ssum)
                rs = sm.tile([1, 1], F32, tag=f"rs{tagp}")
                nc.vector.reciprocal(rs, ssum)
                ones_s = sm.tile([1, 128], F16, tag=f"ones_s{tagp}")
                nc.vector.tensor_scalar_mul(ones_s, ones_row, rs)
                ab_ps = psp.tile([128, n], F32, tag="ps512", bufs=4, name=f"abps{tagp}{b}")
                nc.tensor.matmul(ab_ps, ones_s, ex, start=True, stop=True)
                ab = sm.tile([128, n], F16, tag=f"ab{tagp}")
                nc.scalar.copy(ab, ab_ps)
                return ab

            tail_fn = [None]

            for b in range(nb):
                qth, qtl, vth, vtl = pre0 if b == 0 else load_inputs(b)

                # ---- phase 1+2 interleaved: Bv = Wb V^T, C = Q Bv (3-pass) ----
                bv_hi = mid.tile([128, KD, NV], F16, tag="bv_hi")
                bv_lo = mid.tile([128, KD, NV], F16, tag="bv_lo")
                c_ps = [psp.tile([128, NV], F32, tag="ps196", name=f"c_ps{b}_{m}")
                        for m in range(MQ)]

                def emit_bv(md):
                    pb = psp.tile([128, NV], F32, tag="ps512", bufs=4, name=f"pb{b}_{md}")
                    passes = ((wbth, vth), (wbth, vtl), (wbtl, vth))
                    ds = slice(md * 128, (md + 1) * 128)
                    n = 0
                    for k in range(KD):
                        for lh, rh in passes:
                            n += 1
                            nc.tensor.matmul(pb, lh[:, k, ds], rh[:, k, :],
                                             start=(n == 1), stop=(n == 3 * KD))
                    nc.vector.tensor_copy(bv_hi[:, md, :], pb)
                    nc.vector.tensor_sub(bv_lo[:, md, :], pb, bv_hi[:, md, :])

                def emit_c(k):
                    for m in range(MQ):
                        ms = slice(m * 128, (m + 1) * 128)
                        for i, (lh, rh) in enumerate(((qth, bv_hi), (qtl, bv_hi), (qth, bv_lo))):
                            nc.tensor.matmul(c_ps[m], lh[:, k, ms], rh[:, k, :],
                                             start=(k == 0 and i == 0),
                                             stop=(k == KD - 1 and i == 2))

                for e in range(KD + 1):
                    if e < KD:
                        emit_bv(e)
                    if e == 3 and tail_fn[0] is not None:
                        tail_fn[0]()          # batch b-1 softmax_q + outputs
                        tail_fn[0] = None
                    if e >= 1:
                        emit_c(e - 1)

                c_sb = mid.tile([128, MQ, NV], F16, tag="c")
                for m in range(MQ):
                    nc.scalar.activation(c_sb[:, m, :], c_ps[m], AF.Tanh)

                # ---- phase 3: WqQT, WvVT (3-pass, split hi/lo) ----
                wqqt_hi = mid.tile([128, MQ, D], F16, tag="wqqt_hi")
                wqqt_lo = mid.tile([128, MQ, D], F16, tag="wqqt_lo")
                for m in range(MQ):
                    ms = slice(m * 128, (m + 1) * 128)
                    for h in range(2):
                        hs = slice(h * 512, (h + 1) * 512)
                        p = psp.tile([128, 512], F32, tag="ps512", bufs=4, name=f"pq{b}_{m}_{h}")
                        n = 0
                        for k in range(KD):
                            for lh, rh in ((qth, wqth), (qth, wqtl), (qtl, wqth)):
                                n += 1
                                nc.tensor.matmul(p, lh[:, k, ms], rh[:, k, hs],
                                                 start=(n == 1), stop=(n == 3 * KD))
                        nc.vector.tensor_copy(wqqt_hi[:, m, hs], p)
                        nc.vector.tensor_sub(wqqt_lo[:, m, hs], p, wqqt_hi[:, m, hs])
                wvvt_hi = mid.tile([128, 2, D], F16, tag="wvvt_hi")
                wvvt_lo = mid.tile([128, 2, D], F16, tag="wvvt_lo")
                for m in range(2):
                    rows = 128 if m == 0 else NV1
                    ms = slice(m * 128, m * 128 + rows)
                    for h in range(2):
                        hs = slice(h * 512, (h + 1) * 512)
                        p = psp.tile([128, 512], F32, tag="ps512", bufs=4, name=f"pv{b}_{m}_{h}")
                        n = 0
                        for k in range(KD):
                            for lh, rh in ((vth, wvth), (vth, wvtl), (vtl, wvth)):
                                n += 1
                                nc.tensor.matmul(p[:rows, :], lh[:, k, ms], rh[:, k, hs],
                                                 start=(n == 1), stop=(n == 3 * KD))
                        nc.vector.tensor_copy(wvvt_hi[:rows, m, hs], p[:rows, :])
                        nc.vector.tensor_sub(wvvt_lo[:rows, m, hs], p[:rows, :],
                                             wvvt_hi[:rows, m, hs])

                # ---- CT via f16 PE transposes (tanh of C overlapped W above) ----
                ct_sb = mid.tile([128, 2, NQ], F16, tag="ct")
                for mv in range(2):
                    rows = 128 if mv == 0 else NV1
                    ctp = psp.tile([128, NQ], F16, tag="ps512", bufs=4, name=f"ctp{b}_{mv}")
                    for mq in range(MQ):
                        nc.tensor.matmul(
                            ctp[:rows, mq * 128:(mq + 1) * 128],
                            c_sb[:, mq, mv * 128:mv * 128 + rows],
                            identh, is_transpose=True,
                            start=(mq == 0), stop=(mq == MQ - 1))
                    nc.scalar.copy(ct_sb[:rows, mv, :], ctp[:rows, :])

                # ---- phase 4: H_v (f16) + h_v [2-row dot] ----
                hv_m_l = [None] * KD
                h_v_ps = psp.tile([1, NV], F32, tag="ps196", name=f"hv_acc{b}")

                def emit_hv(m):
                    ms = slice(m * 128, (m + 1) * 128)
                    t2 = psp.tile([128, NV], F32, tag="ps196", name=f"hv2_{b}_{m}")
                    for kq in range(MQ):
                        for i, lh in enumerate((wqqt_hi, wqqt_lo)):
                            nc.tensor.matmul(t2, lh[:, kq, ms], c_sb[:, kq, :],
                                             start=(kq == 0 and i == 0),
                                             stop=(kq == MQ - 1 and i == 1))
                    t1 = psp.tile([128, NV], F16, tag="ps196", name=f"hv1_{b}_{m}")
                    nc.tensor.matmul(t1[:, 0:128], wvvt_hi[:, 0, ms], identh,
                                     is_transpose=True, start=True, stop=False)
                    nc.tensor.matmul(t1[:, 128:NV], wvvt_hi[:NV1, 1, ms],
                                     identh[:NV1, :NV1],
                                     is_transpose=True, start=False, stop=True)
                    t1sb = sm.tile([128, NV], F16, tag="t1v", bufs=2, name=f"t1v{b}_{m}")
                    nc.scalar.copy(t1sb, t1)
                    pre = sm.tile([128, NV], F32, tag="prev", bufs=2, name=f"prev{b}_{m}")
                    nc.vector.scalar_tensor_tensor(out=pre, in0=t2, scalar=1.0, in1=t1sb,
                                                   op0=ALU.mult, op1=ALU.add)
                    hv_m = sm.tile([128, NV], F16, tag="hvm", bufs=2, name=f"hvm{b}_{m}")
                    nc.scalar.activation(hv_m, pre, AF.Tanh)
                    hv_m_l[m] = hv_m

                def emit_hv_dot(m):
                    nc.tensor.matmul(h_v_ps, whv_sb[:, m, 0:1], hv_m_l[m],
                                     start=(m == 0), stop=False)
                    nc.tensor.matmul(h_v_ps, whv_sb[:, m, 1:2], hv_m_l[m],
                                     start=False, stop=(m == KD - 1))

                for m in range(KD + 1):
                    if m < KD:
                        emit_hv(m)
                    if m >= 1:
                        emit_hv_dot(m - 1)
                # release h_v_ps right away so the psum rotation stays free
                h_v_sb = sm.tile([1, NV], F32, tag="hvsb")
                nc.scalar.copy(h_v_sb, h_v_ps)

                # ---- phase 5: H_q + h_q, with v-softmax/v_hat woven in ----
                hq_m_l = [None] * KD
                h_q_ps = psp.tile([1, NQ], F32, tag="ps512", bufs=4, name=f"hq_acc{b}")
                av_b_box = [None]
                vhat_sb = sm.tile([128, KD], F32, tag="vhat")

                def emit_hq(m):
                    ms = slice(m * 128, (m + 1) * 128)
                    t2 = psp.tile([128, NQ], F32, tag="ps512", bufs=4, name=f"hq2_{b}_{m}")
                    for kv in range(2):
                        rows = 128 if kv == 0 else NV1
                        for i, lh in enumerate((wvvt_hi, wvvt_lo)):
                            nc.tensor.matmul(t2, lh[:rows, kv, ms], ct_sb[:rows, kv, :],
                                             start=(kv == 0 and i == 0),
                                             stop=(kv == 1 and i == 1))
                    t1 = psp.tile([128, NQ], F16, tag="ps512", bufs=4, name=f"hq1_{b}_{m}")
                    for kq in range(MQ):
                        nc.tensor.matmul(t1[:, kq * 128:(kq + 1) * 128],
                                         wqqt_hi[:, kq, ms], identh, is_transpose=True,
                                         start=(kq == 0), stop=(kq == MQ - 1))
                    t1sb = sm.tile([128, NQ], F16, tag="t1q", bufs=2, name=f"t1q{b}_{m}")
                    nc.scalar.copy(t1sb, t1)
                    pre = sm.tile([128, NQ], F32, tag="preq", bufs=2, name=f"preq{b}_{m}")
                    nc.vector.scalar_tensor_tensor(out=pre, in0=t2, scalar=1.0, in1=t1sb,
                                                   op0=ALU.mult, op1=ALU.add)
                    hq_m = sm.tile([128, NQ], F16, tag="hqm", bufs=2, name=f"hqm{b}_{m}")
                    nc.scalar.activation(hq_m, pre, AF.Tanh)
                    hq_m_l[m] = hq_m

                def emit_hq_dot(m):
                    nc.tensor.matmul(h_q_ps, whq_sb[:, m, 0:1], hq_m_l[m],
                                     start=(m == 0), stop=False)
                    nc.tensor.matmul(h_q_ps, whq_sb[:, m, 1:2], hq_m_l[m],
                                     start=False, stop=(m == KD - 1))

                for m in range(KD + 1):
                    if m < KD:
                        emit_hq(m)
                    if m == 3:
                        av_b_box[0] = softmax_bcast(b, h_v_sb, NV, "v")
                    if m == 5:
                        for k in range(KD):
                            nc.vector.scalar_tensor_tensor(
                                out=sm.tile([128, NV], F16, tag="scrv", name="scrv"),
                                in0=vth[:, k, :], scalar=1.0, in1=av_b_box[0],
                                op0=ALU.mult, op1=ALU.mult,
                                accum_out=vhat_sb[:, k:k + 1])
                    if m >= 1:
                        emit_hq_dot(m - 1)
                # release h_q_ps before deferring the q-softmax into batch b+1
                h_q_sb = sm.tile([1, NQ], F32, tag="hqsb")
                nc.scalar.copy(h_q_sb, h_q_ps)

                # ---- deferred tail: q-softmax + q_hat + output DMA ----
                def make_tail(b, h_q_sb, qth, vhat_sb):
                    def tail():
                        aq_b = softmax_bcast(b, h_q_sb, NQ, "q")
                        qhat_sb = sm.tile([128, KD], F32, tag="qhat")
                        for k in range(KD):
                            nc.vector.scalar_tensor_tensor(
                                out=sm.tile([128, NQ], F16, tag="scrq", name="scrq"),
                                in0=qth[:, k, :], scalar=1.0, in1=aq_b,
                                op0=ALU.mult, op1=ALU.mult,
                                accum_out=qhat_sb[:, k:k + 1])
                        nc.sync.dma_start(out=OV_d[b].rearrange("(k p) -> p k", p=128),
                                          in_=vhat_sb)
                        nc.sync.dma_start(out=OQ_d[b].rearrange("(k p) -> p k", p=128),
                                          in_=qhat_sb)
                    return tail

                tail_fn[0] = make_tail(b, h_q_sb, qth, vhat_sb)

            tail_fn[0]()

    nc.finalize()
    return nc


_BUILT = {}


def _split(x):
    hi = x.astype(np.float16)
    lo = (x - hi.astype(np.float32)).astype(np.float16)
    return np.ascontiguousarray(hi), np.ascontiguousarray(lo)


def kernel(V, Q, W_b, W_v, W_q, w_hv, w_hq, _trace=False):
    V = np.asarray(V, dtype=np.float32)
    Q = np.asarray(Q, dtype=np.float32)
    nb = B // NCORES
    QTh, QTl = _split(Q.transpose(0, 2, 1))      # [B, D, NQ] f16
    VTh, VTl = _split(V.transpose(0, 2, 1))      # [B, D, NV] f16
    WbTh, WbTl = _split(np.asarray(W_b, dtype=np.float32).T)
    WqTh, WqTl = _split(np.asarray(W_q, dtype=np.float32).T)
    WvTh, WvTl = _split(np.asarray(W_v, dtype=np.float32).T)

    def whsplit(w):
        w = np.asarray(w, dtype=np.float32).reshape(D)
        hi = w.astype(np.float16)
        lo = (w - hi.astype(np.float32)).astype(np.float16)
        return np.ascontiguousarray(np.stack([hi, lo], axis=-1))  # [D, 2] f16

    whv2 = whsplit(w_hv)
    whq2 = whsplit(w_hq)

    if nb not in _BUILT:
        _BUILT[nb] = build(nb)
    nc = _BUILT[nb]

    in_maps = []
    for c in range(NCORES):
        sl = slice(c * nb, (c + 1) * nb)
        in_maps.append({
            "QTh": np.ascontiguousarray(QTh[sl]), "QTl": np.ascontiguousarray(QTl[sl]),
            "VTh": np.ascontiguousarray(VTh[sl]), "VTl": np.ascontiguousarray(VTl[sl]),
            "WbTh": WbTh, "WbTl": WbTl, "WqTh": WqTh, "WqTl": WqTl,
            "WvTh": WvTh, "WvTl": WvTl, "whv2": whv2, "whq2": whq2,
        })

    out = run_bass_kernel_spmd(nc, in_maps, core_ids=list(range(NCORES)),
                               trace=_trace)
    v_hat = np.concatenate([out.results[c]["OV"] for c in range(NCORES)], axis=0)
    q_hat = np.concatenate([out.results[c]["OQ"] for c in range(NCORES)], axis=0)
    if _trace:
        kernel._last_exec_ns = out.exec_time_ns
        kernel._last_results = out
    return (v_hat, q_hat)


# revision 24
# speedup vs baseline: 1.0929x; 1.0306x over previous
"""CoAttention forward on 8 TRN2 NeuronCores.

Data-parallel over batch B=64 (8 batches/core). All precision-critical
matmuls are 3-pass f16 hi/lo (~22-bit); the softmax logits are extremely
sensitive (pre-tanh affinities have std ~1e3, softmax is near-argmax), so
2-pass is not enough on the affinity/projection paths.

Structure (per batch, Q [512,1024], V [196,1024], D=1024):
  Bv   = W_b V^T               [D, NV]   3-pass, split hi/lo on chip
  C    = tanh(Q Bv)            [NQ, NV]  3-pass, stored f16
         (C = Q (W_b V^T) association halves the affinity-path PE work
          vs (Q W_b) V^T since NV=196 < NQ=512)
  WqQT = Q W_q^T               [NQ, D]   3-pass, split hi/lo
  WvVT = V W_v^T               [NV, D]   3-pass, split hi/lo
  CT   = C^T                   via PE f16 transposes (after W so tanh overlaps)
  H_v  = tanh(transpose(WvVT_hi) + (WqQT_hi + WqQT_lo) C)   f16 store
  h_v  = [whv_hi whv_lo]^T H_v   (M=2 matmul; rows summed on GpSimd)
  a_v  = softmax(h_v); broadcast to 128 partitions via f16 PE matmul
  H_q / h_q / a_q analogous with CT
  v_hat = sum_v a_v[v] VT_hi[:, v];  q_hat = sum_q a_q[q] QT_hi[:, q]

Cross-batch software pipelining: batch b's q-softmax + v_hat/q_hat
reductions + output DMA are emitted inside batch b+1's Bv phase; the
v-softmax is woven into the H_q loop. This keeps the PE from stalling on
the vector/scalar softmax chains at batch boundaries.

kernel(**inputs) takes FULL inputs, shards internally, returns (v_hat, q_hat).
"""
import numpy as np

import concourse.bass as bass
import concourse.mybir as mybir
import concourse.tile as tile
from concourse import bacc, bass_isa
from concourse.bass_utils import run_bass_kernel_spmd
from concourse.masks import make_identity

AF = mybir.ActivationFunctionType
ALU = mybir.AluOpType
AX = mybir.AxisListType
F32 = mybir.dt.float32
F16 = mybir.dt.float16

B, NV, NQ, D = 64, 196, 512, 1024
NCORES = 8
NB = B // NCORES          # batches per core
KD = D // 128             # 8 feature k-tiles
MQ = NQ // 128            # 4 NQ m-tiles
NV1 = NV - 128            # 68 (second NV tile)


def build(nb=NB):
    nc = bacc.Bacc(None, target_bir_lowering=False)

    QTh_d = nc.dram_tensor("QTh", [nb, D, NQ], F16, kind="ExternalInput")
    QTl_d = nc.dram_tensor("QTl", [nb, D, NQ], F16, kind="ExternalInput")
    VTh_d = nc.dram_tensor("VTh", [nb, D, NV], F16, kind="ExternalInput")
    VTl_d = nc.dram_tensor("VTl", [nb, D, NV], F16, kind="ExternalInput")
    WbTh_d = nc.dram_tensor("WbTh", [D, D], F16, kind="ExternalInput")
    WbTl_d = nc.dram_tensor("WbTl", [D, D], F16, kind="ExternalInput")
    WqTh_d = nc.dram_tensor("WqTh", [D, D], F16, kind="ExternalInput")
    WqTl_d = nc.dram_tensor("WqTl", [D, D], F16, kind="ExternalInput")
    WvTh_d = nc.dram_tensor("WvTh", [D, D], F16, kind="ExternalInput")
    WvTl_d = nc.dram_tensor("WvTl", [D, D], F16, kind="ExternalInput")
    whv_d = nc.dram_tensor("whv2", [D, 2], F16, kind="ExternalInput")
    whq_d = nc.dram_tensor("whq2", [D, 2], F16, kind="ExternalInput")
    OV_d = nc.dram_tensor("OV", [nb, D], F32, kind="ExternalOutput")
    OQ_d = nc.dram_tensor("OQ", [nb, D], F32, kind="ExternalOutput")

    with tile.TileContext(nc) as tc:
        with (
            tc.tile_pool(name="wsb", bufs=1) as wsb,
            tc.tile_pool(name="iop", bufs=2) as iop,
            tc.tile_pool(name="mid", bufs=1) as mid,
            tc.tile_pool(name="sm", bufs=1) as sm,
            tc.tile_pool(name="psp", bufs=4, space="PSUM") as psp,
        ):
            def wtile(name, src):
                t = wsb.tile([128, KD, D], F16, name=name)
                nc.sync.dma_start(out=t, in_=src.rearrange("(k p) d -> p k d", p=128))
                return t

            def load_inputs(b):
                # V first: the Bv chains consume it before Q is needed
                def inp(name, src, n):
                    t = iop.tile([128, KD, n], F16, tag=name)
                    nc.sync.dma_start(out=t, in_=src.rearrange("(k p) n -> p k n", p=128))
                    return t

                vth = inp("vth", VTh_d[b], NV)
                vtl = inp("vtl", VTl_d[b], NV)
                qth = inp("qth", QTh_d[b], NQ)
                qtl = inp("qtl", QTl_d[b], NQ)
                return qth, qtl, vth, vtl

            # DMA order = queue order: Bv-critical tensors first, k-groups
            # interleaved across all four so Bv chain 0 can start once the
            # k=0 group lands instead of waiting for whole tensors.
            wbth = wsb.tile([128, KD, D], F16, name="wbth")
            wbtl = wsb.tile([128, KD, D], F16, name="wbtl")
            vth0 = iop.tile([128, KD, NV], F16, tag="vth")
            vtl0 = iop.tile([128, KD, NV], F16, tag="vtl")
            for k in range(KD):
                for t, src in ((wbth, WbTh_d), (wbtl, WbTl_d)):
                    nc.sync.dma_start(out=t[:, k, :],
                                      in_=src.rearrange("(k p) d -> p k d", p=128)[:, k, :])
                for t, src in ((vth0, VTh_d[0]), (vtl0, VTl_d[0])):
                    nc.sync.dma_start(out=t[:, k, :],
                                      in_=src.rearrange("(k p) n -> p k n", p=128)[:, k, :])
            qth0 = iop.tile([128, KD, NQ], F16, tag="qth")
            nc.sync.dma_start(out=qth0, in_=QTh_d[0].rearrange("(k p) n -> p k n", p=128))
            qtl0 = iop.tile([128, KD, NQ], F16, tag="qtl")
            nc.sync.dma_start(out=qtl0, in_=QTl_d[0].rearrange("(k p) n -> p k n", p=128))
            pre0 = (qth0, qtl0, vth0, vtl0)
            wqth = wtile("wqth", WqTh_d)
            wqtl = wtile("wqtl", WqTl_d)
            wvth = wtile("wvth", WvTh_d)
            wvtl = wtile("wvtl", WvTl_d)
            whv_sb = wsb.tile([128, KD, 2], F16)
            nc.sync.dma_start(out=whv_sb, in_=whv_d.rearrange("(k p) t -> p k t", p=128))
            whq_sb = wsb.tile([128, KD, 2], F16)
            nc.sync.dma_start(out=whq_sb, in_=whq_d.rearrange("(k p) t -> p k t", p=128))
            identh = wsb.tile([128, 128], F16)
            make_identity(nc, identh)
            ones_row = wsb.tile([1, 128], F32)
            nc.vector.memset(ones_row, 1.0)

            def softmax_bcast(b, h_sb, n, tagp):
                negm = sm.tile([1, 1], F32, tag=f"negm{tagp}")
                nc.vector.reduce_max(negm, h_sb, axis=AX.X, negate=True)
                ex = sm.tile([1, n], F16, tag=f"ex{tagp}")
                ssum = sm.tile([1, 1], F32, tag=f"ssum{tagp}")
                nc.scalar.activation(ex, h_sb, AF.Exp, bias=negm, accum_out=ssum)
                rs = sm.tile([1, 1], F32, tag=f"rs{tagp}")
                nc.vector.reciprocal(rs, ssum)
                ones_s = sm.tile([1, 128], F16, tag=f"ones_s{tagp}")
                nc.vector.tensor_scalar_mul(ones_s, ones_row, rs)
                ab_ps = psp.tile([128, n], F32, tag="ps512", bufs=4, name=f"abps{tagp}{b}")
                nc.tensor.matmul(ab_ps, ones_s, ex, start=True, stop=True)
                ab = sm.tile([128, n], F16, tag=f"ab{tagp}")
                nc.scalar.copy(ab, ab_ps)
                return ab

            tail_fn = [None]

            for b in range(nb):
                qth, qtl, vth, vtl = pre0 if b == 0 else load_inputs(b)

                # ---- phase 1+2 interleaved: Bv = Wb V^T, C = Q Bv (3-pass) ----
                bv_hi = mid.tile([128, KD, NV], F16, tag="bv_hi")
                bv_lo = mid.tile([128, KD, NV], F16, tag="bv_lo")
                c_ps = [psp.tile([128, NV], F32, tag="ps196", name=f"c_ps{b}_{m}")
                        for m in range(MQ)]

                def emit_bv(md):
                    pb = psp.tile([128, NV], F32, tag="ps512", bufs=4, name=f"pb{b}_{md}")
                    passes = ((wbth, vth), (wbth, vtl), (wbtl, vth))
                    ds = slice(md * 128, (md + 1) * 128)
                    n = 0
                    for k in range(KD):
                        for lh, rh in passes:
                            n += 1
                            nc.tensor.matmul(pb, lh[:, k, ds], rh[:, k, :],
                                             start=(n == 1), stop=(n == 3 * KD))
                    nc.vector.tensor_copy(bv_hi[:, md, :], pb)
                    nc.vector.tensor_sub(bv_lo[:, md, :], pb, bv_hi[:, md, :])

                def emit_c(k):
                    for m in range(MQ):
                        ms = slice(m * 128, (m + 1) * 128)
                        for i, (lh, rh) in enumerate(((qth, bv_hi), (qtl, bv_hi), (qth, bv_lo))):
                            nc.tensor.matmul(c_ps[m], lh[:, k, ms], rh[:, k, :],
                                             start=(k == 0 and i == 0),
                                             stop=(k == KD - 1 and i == 2))

                for e in range(KD + 1):
                    if e < KD:
                        emit_bv(e)
                    if e == 3 and tail_fn[0] is not None:
                        tail_fn[0]()          # batch b-1 softmax_q + outputs
                        tail_fn[0] = None
                    if e >= 1:
                        emit_c(e - 1)

                c_sb = mid.tile([128, MQ, NV], F16, tag="c")
                for m in range(MQ):
                    nc.scalar.activation(c_sb[:, m, :], c_ps[m], AF.Tanh)

                # ---- phase 3: WqQT, WvVT (3-pass, split hi/lo) ----
                wqqt_hi = mid.tile([128, MQ, D], F16, tag="wqqt_hi")
                wqqt_lo = mid.tile([128, MQ, D], F16, tag="wqqt_lo")
                for m in range(MQ):
                    ms = slice(m * 128, (m + 1) * 128)
                    for h in range(2):
                        hs = slice(h * 512, (h + 1) * 512)
                        p = psp.tile([128, 512], F32, tag="ps512", bufs=4, name=f"pq{b}_{m}_{h}")
                        n = 0
                        for k in range(KD):
                            for lh, rh in ((qth, wqth), (qth, wqtl), (qtl, wqth)):
                                n += 1
                                nc.tensor.matmul(p, lh[:, k, ms], rh[:, k, hs],
                                                 start=(n == 1), stop=(n == 3 * KD))
                        nc.vector.tensor_copy(wqqt_hi[:, m, hs], p)
                        nc.vector.tensor_sub(wqqt_lo[:, m, hs], p, wqqt_hi[:, m, hs])
                wvvt_hi = mid.tile([128, 2, D], F16, tag="wvvt_hi")
                wvvt_lo = mid.tile([128, 2, D], F16, tag="wvvt_lo")
                for m in range(2):
                    rows = 128 if m == 0 else NV1
                    ms = slice(m * 128, m * 128 + rows)
                    for h in range(2):
                        hs = slice(h * 512, (h + 1) * 512)
                        p = psp.tile([128, 512], F32, tag="ps512", bufs=4, name=f"pv{b}_{m}_{h}")
                        n = 0
                        for k in range(KD):
                            for lh, rh in ((vth, wvth), (vth, wvtl), (vtl, wvth)):
                                n += 1
                                nc.tensor.matmul(p[:rows, :], lh[:, k, ms], rh[:, k, hs],
                                                 start=(n == 1), stop=(n == 3 * KD))
                        nc.vector.tensor_copy(wvvt_hi[:rows, m, hs], p[:rows, :])
                        nc.vector.tensor_sub(wvvt_lo[:rows, m, hs], p[:rows, :],
                                             wvvt_hi[:rows, m, hs])

                # ---- CT via f16 PE transposes (tanh of C overlapped W above) ----
                ct_sb = mid.tile([128, 2, NQ], F16, tag="ct")
                for mv in range(2):
                    rows = 128 if mv == 0 else NV1
                    ctp = psp.tile([128, NQ], F16, tag="ps512", bufs=4, name=f"ctp{b}_{mv}")
                    for mq in range(MQ):
                        nc.tensor.matmul(
                            ctp[:rows, mq * 128:(mq + 1) * 128],
                            c_sb[:, mq, mv * 128:mv * 128 + rows],
                            identh, is_transpose=True,
                            start=(mq == 0), stop=(mq == MQ - 1))
                    nc.scalar.copy(ct_sb[:rows, mv, :], ctp[:rows, :])

                # ---- phase 4: H_v (f16) + h_v [2-row dot] ----
                hv_m_l = [None] * KD
                h_v_ps = psp.tile([1, NV], F32, tag="ps196", name=f"hv_acc{b}")

                def emit_hv(m):
                    ms = slice(m * 128, (m + 1) * 128)
                    t2 = psp.tile([128, NV], F32, tag="ps196", name=f"hv2_{b}_{m}")
                    for kq in range(MQ):
                        for i, lh in enumerate((wqqt_hi, wqqt_lo)):
                            nc.tensor.matmul(t2, lh[:, kq, ms], c_sb[:, kq, :],
                                             start=(kq == 0 and i == 0),
                                             stop=(kq == MQ - 1 and i == 1))
                    t1 = psp.tile([128, NV], F16, tag="ps196", name=f"hv1_{b}_{m}")
                    nc.tensor.matmul(t1[:, 0:128], wvvt_hi[:, 0, ms], identh,
                                     is_transpose=True, start=True, stop=False)
                    nc.tensor.matmul(t1[:, 128:NV], wvvt_hi[:NV1, 1, ms],
                                     identh[:NV1, :NV1],
                                     is_transpose=True, start=False, stop=True)
                    t1sb = sm.tile([128, NV], F16, tag="t1v", bufs=2, name=f"t1v{b}_{m}")
                    nc.scalar.copy(t1sb, t1)
                    pre = sm.tile([128, NV], F32, tag="prev", bufs=2, name=f"prev{b}_{m}")
                    nc.vector.scalar_tensor_tensor(out=pre, in0=t2, scalar=1.0, in1=t1sb,
                                                   op0=ALU.mult, op1=ALU.add)
                    hv_m = sm.tile([128, NV], F16, tag="hvm", bufs=2, name=f"hvm{b}_{m}")
                    nc.scalar.activation(hv_m, pre, AF.Tanh)
                    hv_m_l[m] = hv_m

                def emit_hv_dot(m):
                    nc.tensor.matmul(h_v_ps, whv_sb[:, m, 0:1], hv_m_l[m],
                                     start=(m == 0), stop=False)
                    nc.tensor.matmul(h_v_ps, whv_sb[:, m, 1:2], hv_m_l[m],
                                     start=False, stop=(m == KD - 1))

                for m in range(KD + 1):
                    if m < KD:
                        emit_hv(m)
                    if m >= 1:
                        emit_hv_dot(m - 1)
                # release h_v_ps right away so the psum rotation stays free
                h_v_sb = sm.tile([1, NV], F32, tag="hvsb")
                nc.scalar.copy(h_v_sb, h_v_ps)

                # ---- phase 5: H_q + h_q, with v-softmax/v_hat woven in ----
                hq_m_l = [None] * KD
                h_q_ps = psp.tile([1, NQ], F32, tag="ps512", bufs=4, name=f"hq_acc{b}")
                av_b_box = [None]
                vhat_sb = sm.tile([128, KD], F32, tag="vhat")

                def emit_hq(m):
                    ms = slice(m * 128, (m + 1) * 128)
                    t2 = psp.tile([128, NQ], F32, tag="ps512", bufs=4, name=f"hq2_{b}_{m}")
                    for kv in range(2):
                        rows = 128 if kv == 0 else NV1
                        for i, lh in enumerate((wvvt_hi, wvvt_lo)):
                            nc.tensor.matmul(t2, lh[:rows, kv, ms], ct_sb[:rows, kv, :],
                                             start=(kv == 0 and i == 0),
                                             stop=(kv == 1 and i == 1))
                    t1 = psp.tile([128, NQ], F16, tag="ps512", bufs=4, name=f"hq1_{b}_{m}")
                    for kq in range(MQ):
                        nc.tensor.matmul(t1[:, kq * 128:(kq + 1) * 128],
                                         wqqt_hi[:, kq, ms], identh, is_transpose=True,
                                         start=(kq == 0), stop=(kq == MQ - 1))
                    t1sb = sm.tile([128, NQ], F16, tag="t1q", bufs=2, name=f"t1q{b}_{m}")
                    nc.scalar.copy(t1sb, t1)
                    pre = sm.tile([128, NQ], F32, tag="preq", bufs=2, name=f"preq{b}_{m}")
                    nc.vector.scalar_tensor_tensor(out=pre, in0=t2, scalar=1.0, in1=t1sb,
                                                   op0=ALU.mult, op1=ALU.add)
                    hq_m = sm.tile([128, NQ], F16, tag="hqm", bufs=2, name=f"hqm{b}_{m}")
                    nc.scalar.activation(hq_m, pre, AF.Tanh)
                    hq_m_l[m] = hq_m

                def emit_hq_dot(m):
                    nc.tensor.matmul(h_q_ps, whq_sb[:, m, 0:1], hq_m_l[m],
                                     start=(m == 0), stop=False)
                    nc.tensor.matmul(h_q_ps, whq_sb[:, m, 1:2], hq_m_l[m],
                                     start=False, stop=(m == KD - 1))

                for m in range(KD + 1):
                    if m < KD:
                        emit_hq(m)
                    if m == 4:
                        av_b_box[0] = softmax_bcast(b, h_v_sb, NV, "v")
                    if m == 5:
                        for k in range(KD):
                            nc.vector.scalar_tensor_tensor(
                                out=sm.tile([128, NV], F16, tag="scrv", name="scrv"),
                                in0=vth[:, k, :], scalar=1.0, in1=av_b_box[0],
                                op0=ALU.mult, op1=ALU.mult,
                                accum_out=vhat_sb[:, k:k + 1])
                    if m >= 1:
                        emit_hq_dot(m - 1)
                # release h_q_ps before deferring the q-softmax into batch b+1
                h_q_sb = sm.tile([1, NQ], F32, tag="hqsb")
                nc.scalar.copy(h_q_sb, h_q_ps)

                # ---- deferred tail: q-softmax + q_hat + output DMA ----
                def make_tail(b, h_q_sb, qth, vhat_sb):
                    def tail():
                        aq_b = softmax_bcast(b, h_q_sb, NQ, "q")
                        qhat_sb = sm.tile([128, KD], F32, tag="qhat")
                        for k in range(KD):
                            nc.vector.scalar_tensor_tensor(
                                out=sm.tile([128, NQ], F16, tag="scrq", name="scrq"),
                                in0=qth[:, k, :], scalar=1.0, in1=aq_b,
                                op0=ALU.mult, op1=ALU.mult,
                                accum_out=qhat_sb[:, k:k + 1])
                        nc.sync.dma_start(out=OV_d[b].rearrange("(k p) -> p k", p=128),
                                          in_=vhat_sb)
                        nc.sync.dma_start(out=OQ_d[b].rearrange("(k p) -> p k", p=128),
                                          in_=qhat_sb)
                    return tail

                tail_fn[0] = make_tail(b, h_q_sb, qth, vhat_sb)

            tail_fn[0]()

    nc.finalize()
    return nc


_BUILT = {}


def _split(x):
    hi = x.astype(np.float16)
    lo = (x - hi.astype(np.float32)).astype(np.float16)
    return np.ascontiguousarray(hi), np.ascontiguousarray(lo)


def kernel(V, Q, W_b, W_v, W_q, w_hv, w_hq, _trace=False):
    V = np.asarray(V, dtype=np.float32)
    Q = np.asarray(Q, dtype=np.float32)
    nb = B // NCORES
    QTh, QTl = _split(Q.transpose(0, 2, 1))      # [B, D, NQ] f16
    VTh, VTl = _split(V.transpose(0, 2, 1))      # [B, D, NV] f16
    WbTh, WbTl = _split(np.asarray(W_b, dtype=np.float32).T)
    WqTh, WqTl = _split(np.asarray(W_q, dtype=np.float32).T)
    WvTh, WvTl = _split(np.asarray(W_v, dtype=np.float32).T)

    def whsplit(w):
        w = np.asarray(w, dtype=np.float32).reshape(D)
        hi = w.astype(np.float16)
        lo = (w - hi.astype(np.float32)).astype(np.float16)
        return np.ascontiguousarray(np.stack([hi, lo], axis=-1))  # [D, 2] f16

    whv2 = whsplit(w_hv)
    whq2 = whsplit(w_hq)

    if nb not in _BUILT:
        _BUILT[nb] = build(nb)
    nc = _BUILT[nb]

    in_maps = []
    for c in range(NCORES):
        sl = slice(c * nb, (c + 1) * nb)
        in_maps.append({
            "QTh": np.ascontiguousarray(QTh[sl]), "QTl": np.ascontiguousarray(QTl[sl]),
            "VTh": np.ascontiguousarray(VTh[sl]), "VTl": np.ascontiguousarray(VTl[sl]),
            "WbTh": WbTh, "WbTl": WbTl, "WqTh": WqTh, "WqTl": WqTl,
            "WvTh": WvTh, "WvTl": WvTl, "whv2": whv2, "whq2": whq2,
        })

    out = run_bass_kernel_spmd(nc, in_maps, core_ids=list(range(NCORES)),
                               trace=_trace)
    v_hat = np.concatenate([out.results[c]["OV"] for c in range(NCORES)], axis=0)
    q_hat = np.concatenate([out.results[c]["OQ"] for c in range(NCORES)], axis=0)
    if _trace:
        kernel._last_exec_ns = out.exec_time_ns
        kernel._last_results = out
    return (v_hat, q_hat)
